# revision 22
# baseline (speedup 1.0000x reference)
"""Causal single-head attention (B=4, S=2048, d=1024) on 8 TRN2 NeuronCores.

Sharding: core c -> batch b = c//2, subset s = c%2. Per batch the 16
query blocks (128 rows) are assigned in balanced causal pairs: core
(b,s) owns pairs (lo_i, hi_i) = (2i+s, 15-2i-s), i=0..3, giving every
core 68 true causal score tiles (padded to a uniform 72). Every core
runs the identical instruction stream; causal boundaries come from
per-core 0/1 mask tiles supplied as input data.

K/V projections are tensor-parallel within each core pair: core (b,s)
computes the d_out-half s of kT (fp8, x32-scaled) and v (bf16) for the
whole batch; halves are exchanged with pairwise AllGathers
([[0,1],[2,3],[4,5],[6,7]]). The gathers are ~25us latency-bound and
serialize on the CC core, so K's gather goes first (Phase A needs it
~20us before Phase B needs V's).

Precision: projections and AV run in bf16 (fp32 PSUM). Scores run in
fp8e4m3 DoubleRow (256-deep contraction per pass = 2x bf16 FLOPs):
host folds x32 into Wq and Wk so q' = 32q, k' = 32k sit in the fp8
sweet spot; the combined 2^15 scale is removed inside the EXP
activation (exp(s' / 32768)).

qt8 column layout is [lo0 lo1 lo2 lo3 hi0 hi1 hi2 hi3] (128 cols per
block). At key block kb the live lo blocks are the suffix i >= kb//2
(kb < 8) and the live hi blocks are the prefix i < h(kb); both are
contiguous column ranges, so Phase A needs only TWO wide DR matmuls
per (kb, dc-pair) into two PSUM tiles. Phase B chains slice columns
out of the per-kb exp tiles.
"""
import sys

sys.path.insert(0, "/opt/trn_rl_repo")

import ml_dtypes
import numpy as np

import concourse.bass as bass  # noqa: F401
import concourse.mybir as mybir
import concourse.tile as tile
from concourse import bacc
from concourse.bass_utils import run_bass_kernel_spmd

B, S, D = 4, 2048, 1024
DC = D // 128          # 8 contraction chunks
F32 = mybir.dt.float32
BF = mybir.dt.bfloat16
E4 = mybir.dt.float8e4
E4NP = ml_dtypes.float8_e4m3
BFNP = ml_dtypes.bfloat16
DR = mybir.MatmulPerfMode.DoubleRow
EXP = mybir.ActivationFunctionType.Exp
COPY = mybir.ActivationFunctionType.Copy
GROUPS = [[0, 1], [2, 3], [4, 5], [6, 7]]
EXP_SCALE = 1.0 / 32768.0   # q,k both carry x32; scores carry x1024*32


def _i0(kb):
    """First live lo block at key block kb (suffix i0..3), kb < 8."""
    return kb // 2


def _h(kb):
    """Number of live hi blocks at key block kb (prefix 0..h-1)."""
    return min(4, (17 - kb) // 2)


_cache = {}


def build_nc():
    nc = bacc.Bacc("TRN2", target_bir_lowering=False, debug=False, num_devices=8)
    # inputs, partition-major & contiguous per planned DMA
    xT = nc.dram_tensor("xT", [128, 4, 2, 4, 512], BF, kind="ExternalInput")
    xTq = nc.dram_tensor("xTq", [128, DC, 1024], BF, kind="ExternalInput")
    WqT = nc.dram_tensor("WqT", [128, DC, D], BF, kind="ExternalInput")
    WkTh = nc.dram_tensor("WkTh", [128, 4, DC, 128], BF, kind="ExternalInput")
    WvTh = nc.dram_tensor("WvTh", [128, 2, 4, 512], BF, kind="ExternalInput")
    masks_lo = nc.dram_tensor("masks_lo", [128, 8, 512], BF, kind="ExternalInput")
    masks_hi = nc.dram_tensor("masks_hi", [128, 8, 512], BF, kind="ExternalInput")
    out0 = nc.dram_tensor("out0", [1024, 512], F32, kind="ExternalOutput")
    out1 = nc.dram_tensor("out1", [1024, 512], F32, kind="ExternalOutput")
    # collective buffers
    kg_in = [
        nc.dram_tensor(f"kg_in{h}", [128, 4, 1024], E4) for h in range(2)
    ]
    kg_out = [
        nc.dram_tensor(f"kg_out{h}", [2, 128, 4, 1024], E4) for h in range(2)
    ]
    vg_in = [
        nc.dram_tensor(f"vg_in{h}", [128, 8, 512], BF) for h in range(2)
    ]
    vg_out = [
        nc.dram_tensor(f"vg_out{h}", [2, 128, 8, 512], BF) for h in range(2)
    ]

    with tile.TileContext(nc) as tc:
        with (
            tc.tile_pool(name="w", bufs=1) as wp,
            tc.tile_pool(name="xs", bufs=1) as xsp,
            tc.tile_pool(name="per", bufs=1) as per,
            tc.tile_pool(name="pt", bufs=1) as ptp,
            tc.tile_pool(name="ot", bufs=2) as otp,
            tc.tile_pool(name="sml", bufs=4) as smlp,
            tc.tile_pool(name="mix", bufs=4, space="PSUM") as mixp,
            tc.tile_pool(name="psav", bufs=4, space="PSUM") as psavp,
        ):
            # ---------------- consts + persistent ----------------
            zeros_f = per.tile([128, 2], F32)
            ones = per.tile([128, 2], BF)
            nc.vector.memset(zeros_f, 0.0)
            # exp(0)=1 -> also preloads the ACT exp table long before A
            nc.scalar.activation(ones, zeros_f, EXP)

            # fine-grained head DMAs: first K-proj group needs wk_0 + xs_0_0
            wk = [wp.tile([128, DC, 128], BF, name=f"wk_{o}") for o in range(4)]
            xs = [
                [xsp.tile([128, 4, 512], BF, name=f"xs_{sc}_{h}") for h in range(2)]
                for sc in range(4)
            ]
            nc.sync.dma_start(out=wk[0], in_=WkTh[:, 0])
            nc.sync.dma_start(out=xs[0][0], in_=xT[:, 0, 0])
            nc.sync.dma_start(out=xs[0][1], in_=xT[:, 0, 1])
            for o in range(1, 4):
                nc.sync.dma_start(out=wk[o], in_=WkTh[:, o])
            for h in range(2):
                nc.sync.dma_start(out=xs[1][h], in_=xT[:, 1, h])
            wv = [wp.tile([128, 4, 512], BF, name=f"wv_{h}") for h in range(2)]
            for h in range(2):
                nc.sync.dma_start(out=wv[h], in_=WvTh[:, h])
            for sc in range(2, 4):
                for h in range(2):
                    nc.sync.dma_start(out=xs[sc][h], in_=xT[:, sc, h])
            maskt_lo = per.tile([128, 8, 512], BF)
            maskt_hi = per.tile([128, 8, 512], BF)
            nc.sync.dma_start(out=maskt_lo, in_=masks_lo[:])
            nc.sync.dma_start(out=maskt_hi, in_=masks_hi[:])
            wq = wp.tile([128, DC, D], BF)
            xq = wp.tile([128, DC, 1024], BF)
            nc.sync.dma_start(out=wq, in_=WqT[:])
            nc.sync.dma_start(out=xq, in_=xTq[:])

            # ---- P1/P2 interleaved by key-half: K(0:1024) -> AG_K1,
            # V(0:1024) -> AG_V1, K(1024:) -> AG_K2, V(1024:) -> AG_V2.
            # Four 1MB collectives pipeline on the CC core right after its
            # ~50us init window, each input ready as the previous transfer
            # finishes, so all four land well before their deadlines.
            kg_sb = [
                per.tile([128, 4, 1024], E4, name=f"kg_sb{h}") for h in range(2)
            ]
            vg_sb = [
                per.tile([128, 8, 512], BF, name=f"vg_sb{h}") for h in range(2)
            ]

            def k_half(h):
                for sc in (2 * h, 2 * h + 1):
                    for ocl in range(4):
                        ps = mixp.tile([128, 512], F32, tag="mix",
                                       name=f"ps1_{sc}_{ocl}")
                        for dc in range(DC):
                            nc.tensor.matmul(
                                ps,
                                lhsT=wk[ocl][:, dc, :],
                                rhs=xs[sc][dc // 4][:, dc % 4, :],
                                start=(dc == 0),
                                stop=(dc == DC - 1),
                            )
                        nc.vector.tensor_copy(
                            kg_sb[h][:, ocl, (sc % 2) * 512:(sc % 2) * 512 + 512],
                            ps,
                        )
                nc.scalar.dma_start(out=kg_in[h][:], in_=kg_sb[h])
                nc.gpsimd.collective_compute(
                    "AllGather",
                    mybir.AluOpType.bypass,
                    replica_groups=GROUPS,
                    ins=[kg_in[h][:]],
                    outs=[kg_out[h][:]],
                )

            def v_half(h):
                for sc in (2 * h, 2 * h + 1):
                    for sb in range(4):
                        ps = mixp.tile([128, 512], F32, tag="mix",
                                       name=f"ps2_{sc}_{sb}")
                        for dc in range(DC):
                            nc.tensor.matmul(
                                ps,
                                lhsT=xs[sc][dc // 4][:, dc % 4,
                                                     sb * 128:(sb + 1) * 128],
                                rhs=wv[dc // 4][:, dc % 4, :],
                                start=(dc == 0),
                                stop=(dc == DC - 1),
                            )
                        nc.vector.tensor_copy(
                            vg_sb[h][:, (sc % 2) * 4 + sb, :], ps
                        )
                nc.scalar.dma_start(out=vg_in[h][:], in_=vg_sb[h])
                nc.gpsimd.collective_compute(
                    "AllGather",
                    mybir.AluOpType.bypass,
                    replica_groups=GROUPS,
                    ins=[vg_in[h][:]],
                    outs=[vg_out[h][:]],
                )

            k_half(0)
            v_half(0)
            k_half(1)
            v_half(1)

            # -------- load gathered kt8 (key-halves so A starts early) ---
            kt8 = [
                [wp.tile([128, 4, 1024], E4, name=f"kt8_{ch}_{r}") for r in range(2)]
                for ch in range(2)
            ]
            for ch in range(2):
                for r in range(2):
                    nc.sync.dma_start(out=kt8[ch][r], in_=kg_out[ch][r])

            # -------- P0: Q projection -> qt8 (fp8, overlaps gathers) ----
            qt8 = per.tile([128, DC, 1024], E4)
            for oc in range(8):
                pss = [
                    mixp.tile([128, 512], F32, tag="mix", name=f"ps0_{oc}_{i}")
                    for i in range(2)
                ]
                for dc in range(DC):
                    for sc in range(2):
                        nc.tensor.matmul(
                            pss[sc],
                            lhsT=wq[:, dc, oc * 128:(oc + 1) * 128],
                            rhs=xq[:, dc, sc * 512:(sc + 1) * 512],
                            start=(dc == 0),
                            stop=(dc == DC - 1),
                        )
                for sc in range(2):
                    nc.vector.tensor_copy(
                        qt8[:, oc, sc * 512:(sc + 1) * 512], pss[sc]
                    )

            # -------- load gathered vv (key-halves; lo chains only need a)
            vv = [
                [per.tile([128, 8, 512], BF, name=f"vv_{hf}_{r}") for r in range(2)]
                for hf in range(2)
            ]
            for hf in range(2):
                for r in range(2):
                    for g in range(4):
                        nc.sync.dma_start(
                            out=vv[hf][r][:, 2 * g:2 * g + 2, :],
                            in_=vg_out[hf][r][:, 2 * g:2 * g + 2, :],
                        )

            def vv_rhs(oh, kb):
                return vv[kb // 8][oh][:, kb % 8, :]

            # ------- Phase A: scoresT + exp + mask, two wide tiles per kb
            pt_lo = {}
            pt_hi = {}
            for kb in range(16):
                ktc = kt8[kb // 8]
                kcol = (kb % 8) * 128
                h = _h(kb)
                tiles = [("hi", 512, 128 * h)]
                if kb < 8:
                    i0 = _i0(kb)
                    tiles.append(("lo", 128 * i0, 128 * (4 - i0)))
                pss = {}
                for kind, qo, qw in tiles:
                    pss[kind] = mixp.tile(
                        [128, 512], F32, tag="mix", name=f"psA_{kb}_{kind}"
                    )
                for j in range(4):
                    for kind, qo, qw in tiles:
                        nc.tensor.matmul(
                            pss[kind][:, 0:qw],
                            lhsT=ktc[j // 2][:, (2 * j) % 4:(2 * j) % 4 + 2,
                                             kcol:kcol + 128],
                            rhs=qt8[:, 2 * j:2 * j + 2, qo:qo + qw],
                            start=(j == 0),
                            stop=(j == 3),
                            perf_mode=DR,
                        )
                for kind, qo, qw in tiles:
                    pt = ptp.tile([128, qw], BF, name=f"pt_{kind}_{kb}")
                    nc.scalar.activation(pt, pss[kind][:, 0:qw], EXP,
                                         scale=EXP_SCALE)
                    if kind == "lo":
                        nc.vector.tensor_mul(pt, pt, maskt_lo[:, kb, 0:qw])
                        pt_lo[kb] = pt
                    else:
                        if kb >= 8:
                            nc.vector.tensor_mul(
                                pt, pt, maskt_hi[:, kb - 8, 0:qw]
                            )
                        pt_hi[kb] = pt

            # ---------------- Phase B: chains ----------------
            # lo chains first (need only vva), then hi chains.
            chains = []
            for i in (3, 2, 1, 0):
                chains.append([
                    (pt_lo[kb], 128 * (i - _i0(kb)), kb)
                    for kb in range(0, 2 * i + 2)
                ])
            for i in range(4):
                chains.append([
                    (pt_hi[kb], 128 * i, kb) for kb in range(0, 16 - 2 * i)
                ])

            for ci, tiles in enumerate(chains):
                avs = [
                    psavp.tile([128, 512], F32, tag="psav", name=f"av_{ci}_{oh}")
                    for oh in range(2)
                ]
                lps = psavp.tile([128, 2], F32, tag="psav", name=f"l_{ci}")
                n = len(tiles)
                for idx, (pt, qo, kb) in enumerate(tiles):
                    first, last = idx == 0, idx == n - 1
                    for oh in range(2):
                        nc.tensor.matmul(
                            avs[oh],
                            lhsT=pt[:, qo:qo + 128],
                            rhs=vv_rhs(oh, kb),
                            start=first,
                            stop=last,
                        )
                    nc.tensor.matmul(
                        lps, lhsT=pt[:, qo:qo + 128], rhs=ones,
                        start=first, stop=last,
                    )
                rec = smlp.tile([128, 1], F32, tag="rec")
                nc.vector.reciprocal(rec, lps[:, 0:1])
                ot = otp.tile([128, D], F32, tag="ot")
                nc.vector.tensor_scalar_mul(ot[:, 0:512], avs[0], rec)
                nc.scalar.activation(ot[:, 512:1024], avs[1], COPY, scale=rec)
                nc.scalar.dma_start(
                    out=out0[ci * 128:(ci + 1) * 128, :], in_=ot[:, 0:512]
                )
                nc.sync.dma_start(
                    out=out1[ci * 128:(ci + 1) * 128, :], in_=ot[:, 512:1024]
                )
    nc.compile()
    return nc


def _pair_blocks(sub):
    """(lo_i, hi_i) query-block ids for pairs i=0..3."""
    return [(2 * i + sub, 15 - 2 * i - sub) for i in range(4)]


def _query_cols(sub):
    """qt/xTq column order: [lo0 lo1 lo2 lo3 hi0 hi1 hi2 hi3] x 128."""
    pb = _pair_blocks(sub)
    cols = [np.arange(lo * 128, lo * 128 + 128) for lo, _ in pb]
    cols += [np.arange(hi * 128, hi * 128 + 128) for _, hi in pb]
    return np.concatenate(cols)


def _chain_blocks(sub):
    """Output row order: chains lo3..lo0 then hi0..hi3."""
    pb = _pair_blocks(sub)
    return [pb[i][0] for i in (3, 2, 1, 0)] + [pb[i][1] for i in range(4)]


def _masks(sub):
    """masks_lo / masks_hi [128, 8, 512]: slot kb (lo) / kb-8 (hi)."""
    p = np.arange(128)[:, None]
    j = np.arange(128)[None, :]
    pb = _pair_blocks(sub)
    mlo = np.ones((8, 128, 512), np.float32)
    mhi = np.ones((8, 128, 512), np.float32)
    for kb in range(8):
        i0 = _i0(kb)
        for c, i in enumerate(range(i0, 4)):
            qb = pb[i][0]
            mlo[kb, :, c * 128:(c + 1) * 128] = (kb * 128 + p <= qb * 128 + j)
    for kb in range(8, 16):
        for i in range(_h(kb)):
            qb = pb[i][1]
            mhi[kb - 8, :, i * 128:(i + 1) * 128] = (
                kb * 128 + p <= qb * 128 + j
            )
    return (
        np.ascontiguousarray(mlo.transpose(1, 0, 2)),
        np.ascontiguousarray(mhi.transpose(1, 0, 2)),
    )


def _pmaj(a):
    """[dc*128, cols] -> partition-major [128, dc, cols]."""
    d, cols = a.shape
    return np.ascontiguousarray(a.reshape(d // 128, 128, cols).transpose(1, 0, 2))


def kernel(x, Wq, Wk, Wv, _trace=False):
    if "nc" not in _cache:
        _cache["nc"] = build_nc()
    nc = _cache["nc"]

    x = np.asarray(x, dtype=np.float32)
    # q' = 32q, k' = 32k: x32 into Wq (net of the folded 1/sqrt(d)) and Wk;
    # scores then carry 32*32*32 = 2^15, removed by EXP_SCALE.
    WqTs = _pmaj((np.asarray(Wq, np.float32).T * np.float32(32.0)).astype(BFNP))
    WkTs = (np.asarray(Wk, np.float32).T * np.float32(32.0)).astype(BFNP)
    WvT = np.asarray(Wv, np.float32).T.astype(BFNP)

    in_maps = []
    for c in range(8):
        b, sub = c // 2, c % 2
        xTb = x[b].T.astype(BFNP)                      # [1024, 2048]
        xTp = _pmaj(xTb)                               # [128, 8, 2048]
        xT4 = np.ascontiguousarray(
            xTp.reshape(128, 2, 4, 4, 512).transpose(0, 3, 1, 2, 4)
        )                                              # [128, sc, h, dc%4, 512]
        wkh = _pmaj(WkTs[:, sub * 512:(sub + 1) * 512])  # [128, 8, 512]
        wk4 = np.ascontiguousarray(
            wkh.reshape(128, DC, 4, 128).transpose(0, 2, 1, 3)
        )                                              # [128, 4, 8, 128]
        wvh = _pmaj(WvT[:, sub * 512:(sub + 1) * 512])   # [128, 8, 512]
        wv2 = np.ascontiguousarray(wvh.reshape(128, 2, 4, 512))
        mlo, mhi = _masks(sub)
        in_maps.append(
            {
                "xT": xT4,
                "xTq": _pmaj(np.ascontiguousarray(xTb[:, _query_cols(sub)])),
                "WqT": WqTs,
                "WkTh": wk4,
                "WvTh": wv2,
                "masks_lo": mlo.astype(BFNP),
                "masks_hi": mhi.astype(BFNP),
            }
        )

    res = run_bass_kernel_spmd(
        nc, in_maps, core_ids=list(range(8)), trace=_trace,
        trace_cores=_cache.get("trace_cores"),
    )
    full = np.empty((B, S, D), np.float32)
    for c in range(8):
        b, sub = c // 2, c % 2
        o0, o1 = res.results[c]["out0"], res.results[c]["out1"]
        for pos, qb in enumerate(_chain_blocks(sub)):
            full[b, qb * 128:(qb + 1) * 128, 0:512] = o0[pos * 128:(pos + 1) * 128]
            full[b, qb * 128:(qb + 1) * 128, 512:1024] = o1[pos * 128:(pos + 1) * 128]
    if _trace:
        _cache["last_result"] = res
    return full


# revision 23
# speedup vs baseline: 1.0360x; 1.0360x over previous
"""Causal single-head attention (B=4, S=2048, d=1024) on 8 TRN2 NeuronCores.

Sharding: core c -> batch b = c//2, subset s = c%2. Per batch the 16
query blocks (128 rows) are assigned in balanced causal pairs: core
(b,s) owns pairs (lo_i, hi_i) = (2i+s, 15-2i-s), i=0..3, giving every
core 68 true causal score tiles (padded to a uniform 72). Every core
runs the identical instruction stream; causal boundaries come from
per-core 0/1 mask tiles supplied as input data.

K/V projections are tensor-parallel within each core pair: core (b,s)
computes the d_out-half s of kT (fp8, x32-scaled) and v (bf16) for the
whole batch; halves are exchanged with pairwise AllGathers
([[0,1],[2,3],[4,5],[6,7]]). The gathers are ~25us latency-bound and
serialize on the CC core, so K's gather goes first (Phase A needs it
~20us before Phase B needs V's).

Precision: projections and AV run in bf16 (fp32 PSUM). Scores run in
fp8e4m3 DoubleRow (256-deep contraction per pass = 2x bf16 FLOPs):
host folds x32 into Wq and Wk so q' = 32q, k' = 32k sit in the fp8
sweet spot; the combined 2^15 scale is removed inside the EXP
activation (exp(s' / 32768)).

qt8 column layout is [lo0 lo1 lo2 lo3 hi0 hi1 hi2 hi3] (128 cols per
block). At key block kb the live lo blocks are the suffix i >= kb//2
(kb < 8) and the live hi blocks are the prefix i < h(kb); both are
contiguous column ranges, so Phase A needs only TWO wide DR matmuls
per (kb, dc-pair) into two PSUM tiles. Phase B chains slice columns
out of the per-kb exp tiles.
"""
import sys

sys.path.insert(0, "/opt/trn_rl_repo")

import ml_dtypes
import numpy as np

import concourse.bass as bass  # noqa: F401
import concourse.mybir as mybir
import concourse.tile as tile
from concourse import bacc
from concourse.bass_utils import run_bass_kernel_spmd

B, S, D = 4, 2048, 1024
DC = D // 128          # 8 contraction chunks
F32 = mybir.dt.float32
BF = mybir.dt.bfloat16
E4 = mybir.dt.float8e4
E4NP = ml_dtypes.float8_e4m3
BFNP = ml_dtypes.bfloat16
DR = mybir.MatmulPerfMode.DoubleRow
EXP = mybir.ActivationFunctionType.Exp
COPY = mybir.ActivationFunctionType.Copy
GROUPS = [[0, 1], [2, 3], [4, 5], [6, 7]]
EXP_SCALE = 1.0 / 32768.0   # q,k both carry x32; scores carry x1024*32


def _i0(kb):
    """First live lo block at key block kb (suffix i0..3), kb < 8."""
    return kb // 2


def _h(kb):
    """Number of live hi blocks at key block kb (prefix 0..h-1)."""
    return min(4, (17 - kb) // 2)


_cache = {}


def build_nc():
    nc = bacc.Bacc("TRN2", target_bir_lowering=False, debug=False, num_devices=8)
    # inputs, partition-major & contiguous per planned DMA
    xT = nc.dram_tensor("xT", [128, 4, 2, 4, 512], BF, kind="ExternalInput")
    xTq = nc.dram_tensor("xTq", [128, DC, 1024], BF, kind="ExternalInput")
    WqT = nc.dram_tensor("WqT", [128, DC, D], BF, kind="ExternalInput")
    WkTh = nc.dram_tensor("WkTh", [128, 4, DC, 128], BF, kind="ExternalInput")
    WvTh = nc.dram_tensor("WvTh", [128, 2, 4, 512], BF, kind="ExternalInput")
    masks_lo = nc.dram_tensor("masks_lo", [128, 8, 512], BF, kind="ExternalInput")
    masks_hi = nc.dram_tensor("masks_hi", [128, 8, 512], BF, kind="ExternalInput")
    out0 = nc.dram_tensor("out0", [1024, 512], F32, kind="ExternalOutput")
    out1 = nc.dram_tensor("out1", [1024, 512], F32, kind="ExternalOutput")
    # collective buffers
    kg_in = [
        nc.dram_tensor(f"kg_in{h}", [128, 4, 1024], E4) for h in range(2)
    ]
    kg_out = [
        nc.dram_tensor(f"kg_out{h}", [2, 128, 4, 1024], E4) for h in range(2)
    ]
    vg_in = [
        nc.dram_tensor(f"vg_in{h}", [128, 8, 512], BF) for h in range(2)
    ]
    vg_out = [
        nc.dram_tensor(f"vg_out{h}", [2, 128, 8, 512], BF) for h in range(2)
    ]

    with tile.TileContext(nc) as tc:
        with (
            tc.tile_pool(name="w", bufs=1) as wp,
            tc.tile_pool(name="xs", bufs=1) as xsp,
            tc.tile_pool(name="per", bufs=1) as per,
            tc.tile_pool(name="pt", bufs=1) as ptp,
            tc.tile_pool(name="ot", bufs=2) as otp,
            tc.tile_pool(name="sml", bufs=4) as smlp,
            tc.tile_pool(name="mix", bufs=4, space="PSUM") as mixp,
            tc.tile_pool(name="psav", bufs=4, space="PSUM") as psavp,
        ):
            # ---------------- consts + persistent ----------------
            zeros_f = per.tile([128, 2], F32)
            ones = per.tile([128, 2], BF)
            nc.vector.memset(zeros_f, 0.0)
            # exp(0)=1 -> also preloads the ACT exp table long before A
            nc.scalar.activation(ones, zeros_f, EXP)

            # fine-grained head DMAs: first K-proj group needs wk_0 + xs_0_0
            wk = [wp.tile([128, DC, 128], BF, name=f"wk_{o}") for o in range(4)]
            xs = [
                [xsp.tile([128, 4, 512], BF, name=f"xs_{sc}_{h}") for h in range(2)]
                for sc in range(4)
            ]
            nc.sync.dma_start(out=wk[0], in_=WkTh[:, 0])
            nc.sync.dma_start(out=xs[0][0], in_=xT[:, 0, 0])
            nc.sync.dma_start(out=xs[0][1], in_=xT[:, 0, 1])
            for o in range(1, 4):
                nc.sync.dma_start(out=wk[o], in_=WkTh[:, o])
            for h in range(2):
                nc.sync.dma_start(out=xs[1][h], in_=xT[:, 1, h])
            wv = [wp.tile([128, 4, 512], BF, name=f"wv_{h}") for h in range(2)]
            for h in range(2):
                nc.sync.dma_start(out=wv[h], in_=WvTh[:, h])
            for sc in range(2, 4):
                for h in range(2):
                    nc.sync.dma_start(out=xs[sc][h], in_=xT[:, sc, h])
            maskt_lo = per.tile([128, 8, 512], BF)
            maskt_hi = per.tile([128, 8, 512], BF)
            nc.sync.dma_start(out=maskt_lo, in_=masks_lo[:])
            nc.sync.dma_start(out=maskt_hi, in_=masks_hi[:])
            wq = wp.tile([128, DC, D], BF)
            xq = wp.tile([128, DC, 1024], BF)
            nc.sync.dma_start(out=wq, in_=WqT[:])
            nc.sync.dma_start(out=xq, in_=xTq[:])

            # ---- P1/P2 interleaved by key-half: K(0:1024) -> AG_K1,
            # V(0:1024) -> AG_V1, K(1024:) -> AG_K2, V(1024:) -> AG_V2.
            # Four 1MB collectives pipeline on the CC core right after its
            # ~50us init window, each input ready as the previous transfer
            # finishes, so all four land well before their deadlines.
            kg_sb = [
                per.tile([128, 4, 1024], E4, name=f"kg_sb{h}") for h in range(2)
            ]
            vg_sb = [
                per.tile([128, 8, 512], BF, name=f"vg_sb{h}") for h in range(2)
            ]

            def k_half(h):
                for sc in (2 * h, 2 * h + 1):
                    for ocl in range(4):
                        ps = mixp.tile([128, 512], F32, tag="mix",
                                       name=f"ps1_{sc}_{ocl}")
                        for dc in range(DC):
                            nc.tensor.matmul(
                                ps,
                                lhsT=wk[ocl][:, dc, :],
                                rhs=xs[sc][dc // 4][:, dc % 4, :],
                                start=(dc == 0),
                                stop=(dc == DC - 1),
                            )
                        nc.vector.tensor_copy(
                            kg_sb[h][:, ocl, (sc % 2) * 512:(sc % 2) * 512 + 512],
                            ps,
                        )
                nc.scalar.dma_start(out=kg_in[h][:], in_=kg_sb[h])
                nc.gpsimd.collective_compute(
                    "AllGather",
                    mybir.AluOpType.bypass,
                    replica_groups=GROUPS,
                    ins=[kg_in[h][:]],
                    outs=[kg_out[h][:]],
                )

            def v_half(h):
                for sc in (2 * h, 2 * h + 1):
                    for sb in range(4):
                        ps = mixp.tile([128, 512], F32, tag="mix",
                                       name=f"ps2_{sc}_{sb}")
                        for dc in range(DC):
                            nc.tensor.matmul(
                                ps,
                                lhsT=xs[sc][dc // 4][:, dc % 4,
                                                     sb * 128:(sb + 1) * 128],
                                rhs=wv[dc // 4][:, dc % 4, :],
                                start=(dc == 0),
                                stop=(dc == DC - 1),
                            )
                        nc.vector.tensor_copy(
                            vg_sb[h][:, (sc % 2) * 4 + sb, :], ps
                        )
                nc.scalar.dma_start(out=vg_in[h][:], in_=vg_sb[h])
                nc.gpsimd.collective_compute(
                    "AllGather",
                    mybir.AluOpType.bypass,
                    replica_groups=GROUPS,
                    ins=[vg_in[h][:]],
                    outs=[vg_out[h][:]],
                )

            k_half(0)
            v_half(0)
            k_half(1)
            v_half(1)

            # -------- load gathered kt8 (key-halves so A starts early) ---
            kt8 = [
                [wp.tile([128, 4, 1024], E4, name=f"kt8_{ch}_{r}") for r in range(2)]
                for ch in range(2)
            ]
            for ch in range(2):
                for r in range(2):
                    nc.sync.dma_start(out=kt8[ch][r], in_=kg_out[ch][r])

            # -------- P0: Q projection -> qt8 (fp8, overlaps gathers) ----
            qt8 = per.tile([128, DC, 1024], E4)
            for oc in range(8):
                pss = [
                    mixp.tile([128, 512], F32, tag="mix", name=f"ps0_{oc}_{i}")
                    for i in range(2)
                ]
                for dc in range(DC):
                    for sc in range(2):
                        nc.tensor.matmul(
                            pss[sc],
                            lhsT=wq[:, dc, oc * 128:(oc + 1) * 128],
                            rhs=xq[:, dc, sc * 512:(sc + 1) * 512],
                            start=(dc == 0),
                            stop=(dc == DC - 1),
                        )
                for sc in range(2):
                    nc.vector.tensor_copy(
                        qt8[:, oc, sc * 512:(sc + 1) * 512], pss[sc]
                    )

            # -------- load gathered vv (key-halves; lo chains only need a)
            vv0 = [per.tile([128, 8, 512], BF, name=f"vv0_{r}") for r in range(2)]
            vvq = [
                [per.tile([128, 4, 512], BF, name=f"vvq_{q}_{r}") for r in range(2)]
                for q in range(2)
            ]
            for r in range(2):
                for g in range(4):
                    nc.sync.dma_start(
                        out=vv0[r][:, 2 * g:2 * g + 2, :],
                        in_=vg_out[0][r][:, 2 * g:2 * g + 2, :],
                    )
            for q in range(2):
                for r in range(2):
                    for g in range(2):
                        nc.sync.dma_start(
                            out=vvq[q][r][:, 2 * g:2 * g + 2, :],
                            in_=vg_out[1][r][:, 4 * q + 2 * g:4 * q + 2 * g + 2, :],
                        )

            def vv_rhs(oh, kb):
                if kb < 8:
                    return vv0[oh][:, kb, :]
                return vvq[(kb - 8) // 4][oh][:, (kb - 8) % 4, :]

            # ------- Phase A: scoresT + exp + mask, two wide tiles per kb
            pt_lo = {}
            pt_hi = {}
            for kb in range(16):
                ktc = kt8[kb // 8]
                kcol = (kb % 8) * 128
                h = _h(kb)
                tiles = [("hi", 512, 128 * h)]
                if kb < 8:
                    i0 = _i0(kb)
                    tiles.append(("lo", 128 * i0, 128 * (4 - i0)))
                pss = {}
                for kind, qo, qw in tiles:
                    pss[kind] = mixp.tile(
                        [128, 512], F32, tag="mix", name=f"psA_{kb}_{kind}"
                    )
                for j in range(4):
                    for kind, qo, qw in tiles:
                        nc.tensor.matmul(
                            pss[kind][:, 0:qw],
                            lhsT=ktc[j // 2][:, (2 * j) % 4:(2 * j) % 4 + 2,
                                             kcol:kcol + 128],
                            rhs=qt8[:, 2 * j:2 * j + 2, qo:qo + qw],
                            start=(j == 0),
                            stop=(j == 3),
                            perf_mode=DR,
                        )
                for kind, qo, qw in tiles:
                    pt = ptp.tile([128, qw], BF, name=f"pt_{kind}_{kb}")
                    nc.scalar.activation(pt, pss[kind][:, 0:qw], EXP,
                                         scale=EXP_SCALE)
                    if kind == "lo":
                        nc.vector.tensor_mul(pt, pt, maskt_lo[:, kb, 0:qw])
                        pt_lo[kb] = pt
                    else:
                        if kb >= 8:
                            nc.vector.tensor_mul(
                                pt, pt, maskt_hi[:, kb - 8, 0:qw]
                            )
                        pt_hi[kb] = pt

            # ---------------- Phase B: chains ----------------
            # lo chains first (need only vva), then hi chains.
            chains = []
            for i in (3, 2, 1, 0):
                chains.append([
                    (pt_lo[kb], 128 * (i - _i0(kb)), kb)
                    for kb in range(0, 2 * i + 2)
                ])
            for i in range(4):
                chains.append([
                    (pt_hi[kb], 128 * i, kb) for kb in range(0, 16 - 2 * i)
                ])

            for ci, tiles in enumerate(chains):
                avs = [
                    psavp.tile([128, 512], F32, tag="psav", name=f"av_{ci}_{oh}")
                    for oh in range(2)
                ]
                lps = psavp.tile([128, 2], F32, tag="psav", name=f"l_{ci}")
                n = len(tiles)
                for idx, (pt, qo, kb) in enumerate(tiles):
                    first, last = idx == 0, idx == n - 1
                    for oh in range(2):
                        nc.tensor.matmul(
                            avs[oh],
                            lhsT=pt[:, qo:qo + 128],
                            rhs=vv_rhs(oh, kb),
                            start=first,
                            stop=last,
                        )
                    nc.tensor.matmul(
                        lps, lhsT=pt[:, qo:qo + 128], rhs=ones,
                        start=first, stop=last,
                    )
                rec = smlp.tile([128, 1], F32, tag="rec")
                nc.vector.reciprocal(rec, lps[:, 0:1])
                ot = otp.tile([128, D], F32, tag="ot")
                nc.vector.tensor_scalar_mul(ot[:, 0:512], avs[0], rec)
                nc.scalar.activation(ot[:, 512:1024], avs[1], COPY, scale=rec)
                nc.scalar.dma_start(
                    out=out0[ci * 128:(ci + 1) * 128, :], in_=ot[:, 0:512]
                )
                nc.sync.dma_start(
                    out=out1[ci * 128:(ci + 1) * 128, :], in_=ot[:, 512:1024]
                )
    nc.compile()
    return nc


def _pair_blocks(sub):
    """(lo_i, hi_i) query-block ids for pairs i=0..3."""
    return [(2 * i + sub, 15 - 2 * i - sub) for i in range(4)]


def _query_cols(sub):
    """qt/xTq column order: [lo0 lo1 lo2 lo3 hi0 hi1 hi2 hi3] x 128."""
    pb = _pair_blocks(sub)
    cols = [np.arange(lo * 128, lo * 128 + 128) for lo, _ in pb]
    cols += [np.arange(hi * 128, hi * 128 + 128) for _, hi in pb]
    return np.concatenate(cols)


def _chain_blocks(sub):
    """Output row order: chains lo3..lo0 then hi0..hi3."""
    pb = _pair_blocks(sub)
    return [pb[i][0] for i in (3, 2, 1, 0)] + [pb[i][1] for i in range(4)]


def _masks(sub):
    """masks_lo / masks_hi [128, 8, 512]: slot kb (lo) / kb-8 (hi)."""
    p = np.arange(128)[:, None]
    j = np.arange(128)[None, :]
    pb = _pair_blocks(sub)
    mlo = np.ones((8, 128, 512), np.float32)
    mhi = np.ones((8, 128, 512), np.float32)
    for kb in range(8):
        i0 = _i0(kb)
        for c, i in enumerate(range(i0, 4)):
            qb = pb[i][0]
            mlo[kb, :, c * 128:(c + 1) * 128] = (kb * 128 + p <= qb * 128 + j)
    for kb in range(8, 16):
        for i in range(_h(kb)):
            qb = pb[i][1]
            mhi[kb - 8, :, i * 128:(i + 1) * 128] = (
                kb * 128 + p <= qb * 128 + j
            )
    return (
        np.ascontiguousarray(mlo.transpose(1, 0, 2)),
        np.ascontiguousarray(mhi.transpose(1, 0, 2)),
    )


def _pmaj(a):
    """[dc*128, cols] -> partition-major [128, dc, cols]."""
    d, cols = a.shape
    return np.ascontiguousarray(a.reshape(d // 128, 128, cols).transpose(1, 0, 2))


def kernel(x, Wq, Wk, Wv, _trace=False):
    if "nc" not in _cache:
        _cache["nc"] = build_nc()
    nc = _cache["nc"]

    x = np.asarray(x, dtype=np.float32)
    # q' = 32q, k' = 32k: x32 into Wq (net of the folded 1/sqrt(d)) and Wk;
    # scores then carry 32*32*32 = 2^15, removed by EXP_SCALE.
    WqTs = _pmaj((np.asarray(Wq, np.float32).T * np.float32(32.0)).astype(BFNP))
    WkTs = (np.asarray(Wk, np.float32).T * np.float32(32.0)).astype(BFNP)
    WvT = np.asarray(Wv, np.float32).T.astype(BFNP)

    in_maps = []
    for c in range(8):
        b, sub = c // 2, c % 2
        xTb = x[b].T.astype(BFNP)                      # [1024, 2048]
        xTp = _pmaj(xTb)                               # [128, 8, 2048]
        xT4 = np.ascontiguousarray(
            xTp.reshape(128, 2, 4, 4, 512).transpose(0, 3, 1, 2, 4)
        )                                              # [128, sc, h, dc%4, 512]
        wkh = _pmaj(WkTs[:, sub * 512:(sub + 1) * 512])  # [128, 8, 512]
        wk4 = np.ascontiguousarray(
            wkh.reshape(128, DC, 4, 128).transpose(0, 2, 1, 3)
        )                                              # [128, 4, 8, 128]
        wvh = _pmaj(WvT[:, sub * 512:(sub + 1) * 512])   # [128, 8, 512]
        wv2 = np.ascontiguousarray(wvh.reshape(128, 2, 4, 512))
        mlo, mhi = _masks(sub)
        in_maps.append(
            {
                "xT": xT4,
                "xTq": _pmaj(np.ascontiguousarray(xTb[:, _query_cols(sub)])),
                "WqT": WqTs,
                "WkTh": wk4,
                "WvTh": wv2,
                "masks_lo": mlo.astype(BFNP),
                "masks_hi": mhi.astype(BFNP),
            }
        )

    res = run_bass_kernel_spmd(
        nc, in_maps, core_ids=list(range(8)), trace=_trace,
        trace_cores=_cache.get("trace_cores"),
    )
    full = np.empty((B, S, D), np.float32)
    for c in range(8):
        b, sub = c // 2, c % 2
        o0, o1 = res.results[c]["out0"], res.results[c]["out1"]
        for pos, qb in enumerate(_chain_blocks(sub)):
            full[b, qb * 128:(qb + 1) * 128, 0:512] = o0[pos * 128:(pos + 1) * 128]
            full[b, qb * 128:(qb + 1) * 128, 512:1024] = o1[pos * 128:(pos + 1) * 128]
    if _trace:
        _cache["last_result"] = res
    return full
